# revision 15
# baseline (speedup 1.0000x reference)
"""ConvCapsuleLayer3D Trainium2 kernel.

Sharding: 8 cores = batch(4) x h-half(2). Each core computes a 3D conv
(64->512 ch, 3x3x3, pad 1) over its [64, 16(+2 halo), 32, 32] input slab
as accumulating PE matmuls with output voxels on PSUM partitions, then runs
the 3-iteration capsule routing loop fused in SBUF, and writes
[128 caps, 16, 32*32] activations.

Conv-as-matmul: for each block of 128 voxels (4 w-rows x 32 d) the
stationary operand is a strided view of the padded input slab
[K=(ic, tap), M=128 vox]; the moving operand is the pre-transposed weight
[K, 512 oc]. K-packing: partitions 0-63 hold the slab, 64-127 hold the
slab shifted one h-plane, so one K=128 matmul covers taps (dh,dh+1) of the
same (dw,dd) -> 9 paired K=128 matmuls + 9 single K=64 matmuls for dh=+1.
fp32r at moving-dim 512 runs at 1 cycle/row (4x over plain fp32).
"""
import sys

sys.path.insert(0, "/opt/trn_rl_repo")

from contextlib import ExitStack

import numpy as np

import concourse.bass as bass
import concourse.tile as tile
from concourse import mybir
from concourse.bass_utils import run_bass_kernel_spmd

F32 = mybir.dt.float32
F32R = mybir.dt.float32r

N_CORES = 8
CIN, AIN, COUT, AOUT = 4, 16, 8, 16
IC = CIN * AIN            # 64  conv input channels
OC = CIN * COUT * AOUT    # 512 conv output channels
H = W = D = 32
HP, WP_, DP = 34, 34, 35  # padded slab dims (d padded to 35 for dd+2 reads)
PLANE = WP_ * DP          # 1190 floats per (w,d) plane
PLANES_PER_CORE = 16
SLAB_PLANES = PLANES_PER_CORE + 2
SLAB_F = SLAB_PLANES * PLANE  # 21420
EPS = 1e-8
SIG1 = 0.7310585786300049  # sigmoid(1.0)

_CACHE = {}


def _build_nc(n_planes):
    nc = bass.Bass()
    xa = nc.declare_dram_parameter("xa", [IC, SLAB_F], F32, isOutput=False)
    xb = nc.declare_dram_parameter("xb", [IC, SLAB_F], F32, isOutput=False)
    wp = nc.declare_dram_parameter("wp", [128, 9 * OC], F32, isOutput=False)
    ws = nc.declare_dram_parameter("ws", [IC, 9 * OC], F32, isOutput=False)
    br = nc.declare_dram_parameter("br", [128, 128], F32, isOutput=False)
    ident = nc.declare_dram_parameter("ident", [128, 128], F32, isOutput=False)
    out = nc.declare_dram_parameter(
        "out", [128, PLANES_PER_CORE, 1024], F32, isOutput=True
    )

    taps = [(dw, dd) for dw in (-1, 0, 1) for dd in (-1, 0, 1)]

    with tile.TileContext(nc) as tc, ExitStack() as ctx:
        const = ctx.enter_context(tc.tile_pool(name="const", bufs=1))
        psum = ctx.enter_context(tc.tile_pool(name="psum", bufs=2, space="PSUM"))
        tpsum = ctx.enter_context(tc.tile_pool(name="tpsum", bufs=2, space="PSUM"))
        ring = ctx.enter_context(tc.tile_pool(name="ring", bufs=2))
        scratch = ctx.enter_context(tc.tile_pool(name="scratch", bufs=1))
        small = ctx.enter_context(tc.tile_pool(name="small", bufs=2))

        WPt = const.tile([128, 9 * OC], F32R)
        nc.sync.dma_start(WPt[:, :], wp[:, :].bitcast(F32R))
        WSt = const.tile([IC, 9 * OC], F32R)
        nc.sync.dma_start(WSt[:, :], ws[:, :].bitcast(F32R))
        BR = const.tile([128, 128], F32)
        nc.sync.dma_start(BR[:, :], br[:, :])
        ID = const.tile([128, 128], F32)
        nc.sync.dma_start(ID[:, :], ident[:, :])
        EPSt = const.tile([128, 1], F32)
        nc.vector.memset(EPSt[:, :], EPS)

        WIN_F = 64 + 3 * PLANE + 64
        MAR = 64
        NB = 10
        BLK_OFF = [min(i * 128, PLANE - 128) for i in range(NB)]

        for hl in range(n_planes):
            # sliding 3-plane window: partitions 0-63 = planes (hl..hl+2)
            # of the padded slab, 64-127 = same shifted one plane (hl+1..)
            Wt = ring.tile([128, WIN_F], F32R, tag="window")
            nc.sync.dma_start(
                Wt[0:IC, MAR:MAR + 3 * PLANE],
                xa[:, hl * PLANE:(hl + 3) * PLANE].bitcast(F32R),
            )
            nc.sync.dma_start(
                Wt[IC:128, MAR:MAR + 3 * PLANE],
                xb[:, hl * PLANE:(hl + 3) * PLANE].bitcast(F32R),
            )

            V = ring.tile([128, NB, OC], F32, tag="votes")
            for blk in range(NB):
                o0 = BLK_OFF[blk]
                vp = psum.tile([128, OC], F32, tag="conv")
                for j, (dw, dd) in enumerate(taps):
                    off = MAR + o0 + dw * DP + dd
                    nc.tensor.matmul(
                        vp[:, :],
                        Wt[0:128, off:off + 128],
                        WPt[:, j * OC:(j + 1) * OC],
                        start=(j == 0),
                        stop=False,
                    )
                for j, (dw, dd) in enumerate(taps):
                    off = MAR + 2 * PLANE + o0 + dw * DP + dd
                    nc.tensor.matmul(
                        vp[:, :],
                        Wt[0:IC, off:off + 128],
                        WSt[:, j * OC:(j + 1) * OC],
                        start=False,
                        stop=(j == 8),
                    )
                nc.scalar.copy(V[:, blk, :], vp[:, :])

            # ---- routing over the whole plane (8 blocks x 512 caps) ----
            # free-dim layouts: V (blk, ci, co, ao); P (blk, co, ao);
            # D0/L/R (ci, blk, co); S2/S (blk, co)
            Vv = V[:, :, :]  # [p, 8, 512]
            V_bcico_ao = Vv.rearrange("p b (cico ao) -> p (b cico) ao", ao=AOUT)
            V_bcoao_ci = Vv.rearrange(
                "p b (ci co ao) -> p b (co ao) ci", ci=CIN, co=COUT
            )
            BR_exp = BR[:, :].rearrange(
                "p (one coao) -> p one coao", one=1
            ).broadcast_to([128, NB, 128])

            P = scratch.tile([128, NB, 128], F32, tag="preact")
            A = ring.tile([128, NB, 128], F32, tag="act")
            L = small.tile([128, CIN, NB, COUT], F32, tag="logits")
            R = small.tile([128, CIN, NB, COUT], F32, tag="route")
            VPp = scratch.tile([128, CIN, NB, 128], F32, tag="big")

            for it in range(3):
                if it == 0:
                    # route == sigmoid(1) everywhere: P = SIG1 * sum_ci V + b
                    P0 = scratch.tile([128, NB, 128], F32, tag="p0")
                    nc.vector.tensor_reduce(
                        P0[:, :, :], V_bcoao_ci, mybir.AxisListType.X,
                        mybir.AluOpType.add,
                    )
                    nc.vector.scalar_tensor_tensor(
                        P[:, :, :], P0[:, :, :], SIG1, BR_exp,
                        mybir.AluOpType.mult, mybir.AluOpType.add,
                    )
                else:
                    nc.scalar.activation(
                        R[:, :, :, :], L[:, :, :, :],
                        mybir.ActivationFunctionType.Sigmoid,
                    )
                    RV = scratch.tile([128, NB, OC], F32, tag="rv")
                    for ci in range(CIN):
                        v_ci = Vv.rearrange(
                            "p b (ci co ao) -> p ci b co ao", ci=CIN, co=COUT
                        )[:, ci]
                        rv_ci = RV[:, :, :].rearrange(
                            "p b (ci co ao) -> p ci b co ao", ci=CIN, co=COUT
                        )[:, ci]
                        r_ci = R[:, ci].rearrange(
                            "p b (co one) -> p b co one", one=1
                        ).broadcast_to([128, NB, COUT, AOUT])
                        nc.vector.tensor_tensor(
                            rv_ci, v_ci, r_ci, mybir.AluOpType.mult
                        )
                    RV_red = RV[:, :, :].rearrange(
                        "p b (ci co ao) -> p b (co ao) ci", ci=CIN, co=COUT
                    )
                    P0 = scratch.tile([128, NB, 128], F32, tag="p0")
                    nc.vector.tensor_reduce(
                        P0[:, :, :], RV_red, mybir.AxisListType.X,
                        mybir.AluOpType.add,
                    )
                    nc.vector.tensor_tensor(
                        P[:, :, :], P0[:, :, :], BR_exp, mybir.AluOpType.add
                    )

                # squash scale s = S2 / ((1+S2) * sqrt(S2+eps)) per (blk, co)
                Q = scratch.tile([128, NB, 128], F32, tag="sq")
                nc.scalar.square(Q[:, :, :], P[:, :, :])
                S2 = small.tile([128, NB, COUT], F32, tag="s2")
                nc.vector.tensor_reduce(
                    S2[:, :, :].rearrange("p b co -> p (b co)"),
                    Q[:, :, :].rearrange("p b (co ao) -> p (b co) ao", co=COUT),
                    mybir.AxisListType.X, mybir.AluOpType.add,
                )
                T = small.tile([128, NB, COUT], F32, tag="sqrt")
                nc.scalar.activation(
                    T[:, :, :], S2[:, :, :],
                    mybir.ActivationFunctionType.Sqrt, bias=EPSt[:, :],
                )
                U = small.tile([128, NB, COUT], F32, tag="u")
                nc.vector.tensor_tensor(
                    U[:, :, :], S2[:, :, :], T[:, :, :], mybir.AluOpType.mult
                )
                nc.vector.tensor_tensor(
                    U[:, :, :], U[:, :, :], T[:, :, :], mybir.AluOpType.add
                )
                INV = small.tile([128, NB, COUT], F32, tag="inv")
                nc.vector.reciprocal(INV[:, :, :], U[:, :, :])
                S = small.tile([128, NB, COUT], F32, tag="scale")
                nc.vector.tensor_tensor(
                    S[:, :, :], S2[:, :, :], INV[:, :, :], mybir.AluOpType.mult
                )

                if it < 2:
                    # D0[ci,b,co] = sum_ao V*P ; L += D0 * s
                    for ci in range(CIN):
                        v_ci = Vv.rearrange(
                            "p b (ci co ao) -> p ci b co ao", ci=CIN, co=COUT
                        )[:, ci]
                        p_exp = P[:, :, :].rearrange(
                            "p b (co ao) -> p b co ao", co=COUT
                        )
                        nc.vector.tensor_tensor(
                            VPp[:, ci].rearrange(
                                "p b (co ao) -> p b co ao", co=COUT
                            ),
                            v_ci, p_exp, mybir.AluOpType.mult,
                        )
                    D0 = small.tile([128, CIN, NB, COUT], F32, tag="d0")
                    nc.vector.tensor_reduce(
                        D0[:, :, :, :].rearrange("p ci b co -> p (ci b co)"),
                        VPp[:, :, :, :].rearrange(
                            "p ci b (co ao) -> p (ci b co) ao", co=COUT
                        ),
                        mybir.AxisListType.X, mybir.AluOpType.add,
                    )
                    S_exp = S[:, :, :].rearrange(
                        "p (one b) co -> p one b co", one=1
                    ).broadcast_to([128, CIN, NB, COUT])
                    DS = small.tile([128, CIN, NB, COUT], F32, tag="ds")
                    nc.vector.tensor_tensor(
                        DS[:, :, :, :], D0[:, :, :, :], S_exp,
                        mybir.AluOpType.mult,
                    )
                    if it == 0:
                        nc.vector.tensor_scalar_add(
                            L[:, :, :, :], DS[:, :, :, :], 1.0
                        )
                    else:
                        nc.vector.tensor_tensor(
                            L[:, :, :, :], L[:, :, :, :], DS[:, :, :, :],
                            mybir.AluOpType.add,
                        )
                else:
                    S_exp3 = S[:, :, :].rearrange(
                        "p b (co one) -> p b co one", one=1
                    ).broadcast_to([128, NB, COUT, AOUT])
                    nc.vector.tensor_tensor(
                        A[:, :, :].rearrange(
                            "p b (co ao) -> p b co ao", co=COUT
                        ),
                        P[:, :, :].rearrange(
                            "p b (co ao) -> p b co ao", co=COUT
                        ),
                        S_exp3, mybir.AluOpType.mult,
                    )

            stage = ring.tile([128, PLANE + 128], F32, tag="stage")
            for blk in range(NB):
                tp = tpsum.tile([128, 128], F32, tag="tp")
                nc.tensor.transpose(tp[:, :], A[:, blk, :], ID[:, :])
                nc.scalar.copy(
                    stage[:, BLK_OFF[blk]:BLK_OFF[blk] + 128], tp[:, :]
                )
            valid = stage[:, DP + 1:DP + 1 + 32 * DP].rearrange(
                "p (w d) -> p w d", w=32, d=DP
            )[:, :, 0:32]
            nc.sync.dma_start(
                out[:, hl, :].rearrange("p (w d) -> p w d", w=32, d=32), valid
            )

    _split_wide_waits(nc)
    return nc


def _split_wide_waits(nc, ctrl_limit=1, other_limit=1):
    """walrus codegen caps sync waits per instruction (1 for TPB_CTRL
    Drain/NoOp and Matmult's LW struct, ~3 elsewhere); move excess waits
    onto preceding same-engine NoOps."""
    n_new = 0
    for fn in nc.m.functions:
        for blk in fn.blocks:
            out = []
            for ins in blk.instructions:
                limit = (
                    ctrl_limit
                    if isinstance(
                        ins,
                        (mybir.InstDrain, mybir.InstNoOp, mybir.InstMatmult,
                         mybir.InstLdweights),
                    )
                    else other_limit
                )
                si = ins.sync_info
                if si is not None and si.on_wait and len(si.on_wait) > limit:
                    waits = list(si.on_wait)
                    keep = waits[-limit:]
                    rest = waits[:-limit]
                    step = max(1, ctrl_limit)
                    while rest:
                        chunk, rest = rest[:step], rest[step:]
                        n_new += 1
                        out.append(
                            mybir.InstNoOp(
                                name=f"I-waitsplit-{n_new}",
                                engine=ins.engine,
                                ins=[],
                                outs=[],
                                sync_info=mybir.SyncInfo(
                                    on_wait=chunk, on_update=[]
                                ),
                            )
                        )
                    si.on_wait = keep
                out.append(ins)
            blk.instructions = out
    return n_new


def _host_prep(input_tensor, conv_w, b):
    x = np.asarray(input_tensor, np.float32).reshape(4, IC, H, W, D)
    xpad = np.zeros((4, IC, HP, WP_, DP), np.float32)
    xpad[:, :, 1:33, 1:33, 1:33] = x

    wt = np.ascontiguousarray(
        np.asarray(conv_w, np.float32).transpose(1, 2, 3, 4, 0)
    )  # [ic, dh, dw, dd, oc]
    taps = [(dw, dd) for dw in (-1, 0, 1) for dd in (-1, 0, 1)]
    wp = np.concatenate(
        [
            np.concatenate(
                [wt[:, 0, dw + 1, dd + 1, :], wt[:, 1, dw + 1, dd + 1, :]],
                axis=0,
            )
            for (dw, dd) in taps
        ],
        axis=1,
    )  # [128, 9*512]
    ws = np.concatenate(
        [wt[:, 2, dw + 1, dd + 1, :] for (dw, dd) in taps], axis=1
    )  # [64, 9*512]

    br = np.broadcast_to(
        np.asarray(b, np.float32).reshape(1, 128), (128, 128)
    ).copy()
    ident = np.eye(128, dtype=np.float32)

    in_maps = []
    for c in range(N_CORES):
        bb, hh = c // 2, c % 2
        h0 = hh * PLANES_PER_CORE
        slab = xpad[bb, :, h0:h0 + SLAB_PLANES].reshape(IC, SLAB_F)
        xb_arr = np.zeros_like(slab)
        xb_arr[:, :SLAB_F - PLANE] = slab[:, PLANE:]
        in_maps.append(
            {
                "xa": np.ascontiguousarray(slab),
                "xb": xb_arr,
                "wp": np.ascontiguousarray(wp),
                "ws": np.ascontiguousarray(ws),
                "br": br,
                "ident": ident,
            }
        )
    return in_maps


def kernel(input_tensor, conv_w, b):
    if "nc" not in _CACHE:
        _CACHE["nc"] = _build_nc(PLANES_PER_CORE)
    nc = _CACHE["nc"]
    in_maps = _host_prep(input_tensor, conv_w, b)
    res = run_bass_kernel_spmd(nc, in_maps, list(range(N_CORES)))
    act = np.empty((4, COUT, AOUT, H, W, D), np.float32)
    for c in range(N_CORES):
        bb, hh = c // 2, c % 2
        h0 = hh * PLANES_PER_CORE
        r = res.results[c]["out"].reshape(COUT, AOUT, PLANES_PER_CORE, W, D)
        act[bb, :, :, h0:h0 + PLANES_PER_CORE] = r
    return act
